# revision 2
# baseline (speedup 1.0000x reference)
"""nn_BlockwiseToPixels: per-token MoE routing (16 experts, Linear(256->64)).

Strategy
--------
Routing is per-token, so the token->core assignment is free: each expert's
tokens are dealt evenly across the 8 cores (host-side, from the tiny index
tensor), giving every core near-identical per-expert counts - one shared
SPMD program, no straggler core, and only ~1% padding from rounding segment
capacities to 32. Each core's tokens are shipped grouped by expert and
pre-transposed ([D, ntot]) because the TensorEngine contracts over the
partition axis.

The kernel is HBM-bandwidth bound, so the data path runs in fp16 (the
harness tolerance is 2e-2; fp16 in / fp32 PSUM accumulate / fp16 out
measures ~1e-3): x loads and y stores are half the fp32 bytes. The device
kernel is a static segmented matmul: per 1024-token pair of 512-blocks,
W-stationary fp16 matmul pairs (D=256 split in two K=128 halves)
accumulate in one PSUM bank - the first 512 tokens into partitions 0:64,
the next 512 into partitions 64:128 (col-tiled matmul, tile_position
(0,64)). The expert of every token range is a compile-time constant (the
segment layout), so there is no on-device routing logic. The bias add is
fused into the PSUM->SBUF copy as a full 128-partition op (alternating
vector/scalar engines), and the stacked result ys2 [128, ntot/2] fp16
streams back to HBM at full DMA port width (the host unstacks/unsorts).
Loads are staged in 1 MiB pieces (512-col pieces for the very first group,
issued on both HWDGE rings) so the PE starts early and never starves; the
kernel tail skips Tile's trailing all-engine barrier.

The compiled program depends only on the per-expert segment capacities, so
it is cached across calls.
"""
import os
import sys

sys.path.insert(0, "/opt/trn_rl_repo")

import numpy as np

import concourse.bass as bass
import concourse.mybir as mybir
import concourse.tile as tile
from concourse.bass_utils import run_bass_kernel_spmd

B, T, D, E, P = 32, 8192, 256, 16, 64
N_CORES = 8
BC = B // N_CORES          # batches per core
N_SHARD = BC * T           # tokens per core
GROUP = 8192               # tokens per DMA group
PIECE = 4096               # load piece (cols): 1 MiB fp16 per dma_start

# The pinned walrus accepts only ONE sem wait per instruction, while Tile
# emits instructions carrying several. Hoist extra waits onto InstNoOp
# instructions inserted immediately before, on the same engine (the
# sequencer blocks on each in order - semantically identical).


def _split_multi_waits(nc, max_waits=1):
    n_split = 0
    for f in nc.m.functions:
        for bb in f.blocks:
            il = bb.instructions
            i = 0
            while i < len(il):
                inst = il[i]
                si = inst.sync_info
                if si is not None and si.on_wait and len(si.on_wait) > max_waits:
                    waits = list(si.on_wait)
                    extra, keep = waits[:-max_waits], waits[-max_waits:]
                    nops = []
                    for j, w in enumerate(extra):
                        nop = mybir.InstNoOp(
                            name=f"{inst.name}-waitsplit-{j}", ins=[], outs=[]
                        )
                        nop.engine = inst.engine
                        nop.sync_info = mybir.SyncInfo(on_wait=[w], on_update=[])
                        nops.append(nop)
                    si.on_wait = keep
                    il[i:i] = nops
                    i += len(nops)
                    n_split += 1
                i += 1
    return n_split


class _SlimTileContext(tile.TileContext):
    """TileContext whose kernel tail skips the trailing all-engine barrier.

    The drain instruction already waits on the full vector clock (all
    compute + DMA completions) and the first barrier synchronizes every
    engine behind it. With BASS_KERNEL_TAIL_CLEARS set, semaphores are
    cleared for safe re-execution (needed when timing repeated runs).
    """

    def _drain_and_barrier(self, tick_clock, wait_clock):
        from concourse.tile import ScopedClock

        drain_inst = self.nc.sync.drain()
        wait_clock.add_sem_waits(
            drain_inst.ins, ScopedClock({None: tick_clock.global_clock})
        )
        self.nc.all_engine_barrier()
        popped = self.nc._tile_sem_poison_stack.pop()
        assert popped is self._sem_poison
        if os.environ.get("BASS_KERNEL_TAIL_CLEARS"):
            self.nc.clear_and_free_semaphores(list(self.sems.allocated().values()))


def _segments(caps):
    """Static segment layout helpers for one core."""
    bounds = np.cumsum(np.asarray(caps, dtype=np.int64))

    def expert_at(pos):
        return int(np.searchsorted(bounds, pos, side="right"))

    def runs(s):
        # expert runs within the 512-token block starting at s
        out = []
        pos = s
        while pos < s + 512:
            e = expert_at(pos)
            end = min(s + 512, int(bounds[e]))
            out.append((pos - s, end - pos, e))
            pos = end
        return out

    return expert_at, runs


def _pair_plan(caps):
    """Compile-time plan: per 1024-token pair, the matmul runs for the top
    (partitions 0:64) and bottom (64:128) halves, plus the bias subruns
    with their (expert_top, expert_bottom) combo index."""
    ntot = int(sum(caps))
    expert_at, runs = _segments(caps)
    combos = []
    combo_idx = {}
    plan = []
    for k in range(ntot // 1024):
        s = 1024 * k
        rt = runs(s)
        rb = runs(s + 512)
        cuts = sorted({0, 512} | {o for (o, _, _) in rt} | {o for (o, _, _) in rb})
        subs = []
        for o1, o2 in zip(cuts[:-1], cuts[1:]):
            key = (expert_at(s + o1), expert_at(s + 512 + o1))
            if key not in combo_idx:
                combo_idx[key] = len(combos)
                combos.append(key)
            subs.append((o1, o2 - o1, combo_idx[key]))
        plan.append((rt, rb, subs))
    return plan, combos


def _build_program(caps):
    """Bass program for one core: segmented fp16 matmul over pre-sorted xT.

    caps: tuple of per-expert segment capacities (tokens, multiples of 32);
    their sum (ntot) is a multiple of 1024. Segment boundaries are static.
    """
    ntot = int(sum(caps))
    assert ntot % 1024 == 0
    plan, combos = _pair_plan(caps)
    NC = len(combos)

    # groups of GROUP tokens, with a 1024-multiple tail
    groups = []
    pos = 0
    while pos < ntot:
        gl = min(GROUP, ntot - pos)
        groups.append((pos, gl))
        pos += gl

    nc = bass.Bass(trn_type="TRN2")
    dt = mybir.dt
    xT = nc.declare_dram_parameter("xT", [D, ntot], dt.float16, isOutput=False)
    Wp = nc.declare_dram_parameter("Wp", [128, E * 2 * P], dt.float16, isOutput=False)
    bT = nc.declare_dram_parameter("bT", [128, NC], dt.float32, isOutput=False)
    ys2 = nc.declare_dram_parameter("ys2", [128, ntot // 2], dt.float16, isOutput=True)

    with _SlimTileContext(nc) as tc:
        with (
            tc.tile_pool(name="consts", bufs=1) as consts,
            tc.tile_pool(name="xtp", bufs=3) as xtp,
            tc.tile_pool(name="yp", bufs=3) as yp,
            tc.tile_pool(name="ps", bufs=8, space="PSUM") as ps,
        ):
            # The first matmuls are gated on wt[:, 0:128] + the first x
            # pieces: issue those first, x pieces on the scalar HWDGE ring
            # so they go out in parallel with the W piece on the sync ring.
            wt = consts.tile([128, E * 2 * P], dt.float16)
            first_xt0 = xtp.tile([128, GROUP], dt.float16, tag="xt0")
            first_xt1 = xtp.tile([128, GROUP], dt.float16, tag="xt1")
            nc.sync.dma_start(wt[:, 0:256], Wp[:, 0:256])
            nc.scalar.dma_start(first_xt0[:, 0:512], xT[0:128, 0:512])
            nc.scalar.dma_start(first_xt1[:, 0:512], xT[128:256, 0:512])
            bt = consts.tile([128, NC], dt.float32)
            nc.sync.dma_start(bt[:], bT[:])
            nc.scalar.dma_start(first_xt0[:, 512:1024], xT[0:128, 512:1024])
            nc.scalar.dma_start(first_xt1[:, 512:1024], xT[128:256, 512:1024])
            for s in range(256, E * 2 * P, 1024):
                se = min(s + 1024, E * 2 * P)
                nc.sync.dma_start(wt[:, s:se], Wp[:, s:se])

            pair_i = 0
            for gi, (gof, gl) in enumerate(groups):
                # stage loads in pieces: fine-grained completion lets the PE
                # start on a piece while the rest streams
                if gi == 0:
                    xt0, xt1 = first_xt0, first_xt1
                    pieces = [(1024, 1024), (2048, 2048)]
                    s = 4096
                    while s < gl:
                        pieces.append((s, min(PIECE, gl - s)))
                        s += PIECE
                else:
                    xt0 = xtp.tile([128, GROUP], dt.float16, tag="xt0")
                    xt1 = xtp.tile([128, GROUP], dt.float16, tag="xt1")
                    pieces = [(s, min(PIECE, gl - s)) for s in range(0, gl, PIECE)]
                for s, pl in pieces:
                    nc.sync.dma_start(
                        xt0[:, s : s + pl], xT[0:128, gof + s : gof + s + pl]
                    )
                    nc.sync.dma_start(
                        xt1[:, s : s + pl], xT[128:256, gof + s : gof + s + pl]
                    )

                yts = yp.tile([128, GROUP // 2], dt.float16, tag="yts")
                for kp in range(gl // 1024):
                    rt, rb, subs = plan[pair_i]
                    pair_i += 1
                    q = kp * 1024          # token offset within group
                    yq = kp * 512          # stacked col offset within yts
                    pt = ps.tile([128, 512], dt.float32, tag="pt")
                    for base, rr, xoff in ((0, rt, q), (64, rb, q + 512)):
                        for off, n, e in rr:
                            nc.tensor.matmul(
                                pt[base : base + 64, off : off + n],
                                lhsT=wt[:, (e * 2 + 0) * P : (e * 2 + 1) * P],
                                rhs=xt0[:, xoff + off : xoff + off + n],
                                start=True,
                                stop=False,
                            )
                            nc.tensor.matmul(
                                pt[base : base + 64, off : off + n],
                                lhsT=wt[:, (e * 2 + 1) * P : (e * 2 + 2) * P],
                                rhs=xt1[:, xoff + off : xoff + off + n],
                                start=False,
                                stop=True,
                            )
                    # bias add doubles as the PSUM->SBUF copy; alternate
                    # engines so neither becomes the bottleneck
                    for off, n, j in subs:
                        if pair_i % 2:
                            nc.vector.tensor_scalar_add(
                                yts[:, yq + off : yq + off + n],
                                pt[:, off : off + n],
                                bt[:, j : j + 1],
                            )
                        else:
                            nc.scalar.add(
                                yts[:, yq + off : yq + off + n],
                                pt[:, off : off + n],
                                bt[:, j : j + 1],
                            )
                # stream stores in pieces so no single store issues late;
                # finest pieces on the last group to shrink the kernel tail
                sstep = 1024 if gof + gl == ntot else 2048
                for s in range(0, gl // 2, sstep):
                    pl = min(sstep, gl // 2 - s)
                    nc.scalar.dma_start(
                        ys2[:, gof // 2 + s : gof // 2 + s + pl], yts[:, s : s + pl]
                    )

    return nc, combos


_cache = {"key": None, "nc": None, "combos": None}
last_exec_time_ns = None
_last_run = None


def kernel(x, W, b, block_indices):
    global last_exec_time_ns, _last_run
    x = np.asarray(x, dtype=np.float32)
    W = np.asarray(W, dtype=np.float32)
    b = np.asarray(b, dtype=np.float32)
    sel = np.asarray(block_indices).astype(np.int64).reshape(-1)
    x_flat = x.reshape(B * T, D).astype(np.float16)

    # routing is per-token, so token->core assignment is free: deal each
    # expert's tokens evenly across cores. All cores then have near-identical
    # per-expert counts (no straggler core, minimal shared-layout padding).
    ids = [[None] * E for _ in range(N_CORES)]
    counts = np.zeros((N_CORES, E), dtype=np.int64)
    for e in range(E):
        ge = np.flatnonzero(sel == e)
        parts = np.array_split(ge, N_CORES)
        for c in range(N_CORES):
            ids[c][e] = parts[c]
            counts[c, e] = len(parts[c])

    # shared static segment layout: capacity per expert = max over cores,
    # rounded up to 32; total rounded up to 1024
    caps = ((counts.max(axis=0) + 31) // 32 * 32).astype(np.int64)
    ntot = int(((caps.sum() + 1023) // 1024) * 1024)
    caps[E - 1] += ntot - caps.sum()
    offs = np.concatenate([[0], np.cumsum(caps)])

    key = tuple(int(cp) for cp in caps)
    if _cache["key"] != key:
        nc, combos = _build_program(key)
        _split_multi_waits(nc)
        _cache["nc"] = nc
        _cache["combos"] = combos
        _cache["key"] = key
    combos = _cache["combos"]

    # weights: [E, D, P] -> [128, E*2*P] fp16 tiles (K-half h of expert e at
    # columns (e*2+h)*P); bias as fp32 per-combo stacked columns [128, NC]
    Wp = np.ascontiguousarray(
        W.reshape(E, 2, 128, P).transpose(2, 0, 1, 3).reshape(128, E * 2 * P)
    ).astype(np.float16)
    bT = np.empty((128, len(combos)), dtype=np.float32)
    for j, (eT, eB) in enumerate(combos):
        bT[:64, j] = b[eT]
        bT[64:, j] = b[eB]

    in_maps = []
    for c in range(N_CORES):
        # padded sorted order; pad slots replay token 0 (results discarded)
        po = np.zeros(ntot, dtype=np.int64)
        for e in range(E):
            po[offs[e] : offs[e] + counts[c, e]] = ids[c][e]
        xT = np.ascontiguousarray(x_flat[po].T)
        in_maps.append({"xT": xT, "Wp": Wp, "bT": bT})

    trace = bool(os.environ.get("BASS_KERNEL_TRACE"))
    res = run_bass_kernel_spmd(
        _cache["nc"], in_maps, list(range(N_CORES)), trace=trace
    )
    last_exec_time_ns = res.exec_time_ns
    _last_run = {"nc": _cache["nc"], "in_maps": in_maps}

    out_flat = np.empty((B * T, P), dtype=np.float32)
    npairs = ntot // 1024
    for c in range(N_CORES):
        y2 = res.results[c]["ys2"]  # [128, ntot//2] fp16, stacked pairs
        ys = np.empty((ntot, P), dtype=np.float32)
        ysr = ys.reshape(npairs, 2, 512, P)
        ysr[:, 0] = y2[:64].reshape(64, npairs, 512).transpose(1, 2, 0)
        ysr[:, 1] = y2[64:].reshape(64, npairs, 512).transpose(1, 2, 0)
        for e in range(E):
            out_flat[ids[c][e]] = ys[offs[e] : offs[e] + counts[c, e]]
    return out_flat.reshape(B, T, P)


# revision 8
# speedup vs baseline: 1.1315x; 1.1315x over previous
"""nn_BlockwiseToPixels: per-token MoE routing (16 experts, Linear(256->64)).

Strategy
--------
Routing is per-token, so the token->core assignment is free: each expert's
tokens are dealt evenly across the 8 cores (host-side, from the tiny index
tensor), giving every core near-identical per-expert counts - one shared
SPMD program, no straggler core, and only ~1% padding from rounding segment
capacities to 32. Each core's tokens are shipped grouped by expert and
pre-transposed ([D, ntot]) because the TensorEngine contracts over the
partition axis.

The kernel is HBM-bandwidth bound, so the data path runs in fp16 (the
harness tolerance is 2e-2; fp16 in / fp32 PSUM accumulate / fp16 out
measures ~1e-3): x loads and y stores are half the fp32 bytes. The device
kernel is a static segmented matmul: per 1024-token pair of 512-blocks,
W-stationary fp16 matmul pairs (D=256 split in two K=128 halves)
accumulate in one PSUM bank - the first 512 tokens into partitions 0:64,
the next 512 into partitions 64:128 (col-tiled matmul, tile_position
(0,64)). The expert of every token range is a compile-time constant (the
segment layout), so there is no on-device routing logic. The bias add is
fused into the PSUM->SBUF copy as a full 128-partition op (alternating
vector/scalar engines), and the stacked result ys2 [128, ntot/2] fp16
streams back to HBM at full DMA port width (the host unstacks/unsorts).
Loads are staged in 1 MiB pieces (512-col pieces for the very first group,
issued on both HWDGE rings) so the PE starts early and never starves; the
kernel tail skips Tile's trailing all-engine barrier.

The compiled program depends only on the per-expert segment capacities, so
it is cached across calls.
"""
import os
import sys

sys.path.insert(0, "/opt/trn_rl_repo")

import numpy as np

import concourse.bass as bass
import concourse.mybir as mybir
import concourse.tile as tile
from concourse.bass_utils import run_bass_kernel_spmd

B, T, D, E, P = 32, 8192, 256, 16, 64
N_CORES = 8
BC = B // N_CORES          # batches per core
N_SHARD = BC * T           # tokens per core
GROUP = 8192               # tokens per DMA group
PIECE = 4096               # load piece (cols): 1 MiB fp16 per dma_start

# The pinned walrus accepts only ONE sem wait per instruction, while Tile
# emits instructions carrying several. Hoist extra waits onto InstNoOp
# instructions inserted immediately before, on the same engine (the
# sequencer blocks on each in order - semantically identical).


def _split_multi_waits(nc, max_waits=1):
    n_split = 0
    for f in nc.m.functions:
        for bb in f.blocks:
            il = bb.instructions
            i = 0
            while i < len(il):
                inst = il[i]
                si = inst.sync_info
                if si is not None and si.on_wait and len(si.on_wait) > max_waits:
                    waits = list(si.on_wait)
                    extra, keep = waits[:-max_waits], waits[-max_waits:]
                    nops = []
                    for j, w in enumerate(extra):
                        nop = mybir.InstNoOp(
                            name=f"{inst.name}-waitsplit-{j}", ins=[], outs=[]
                        )
                        nop.engine = inst.engine
                        nop.sync_info = mybir.SyncInfo(on_wait=[w], on_update=[])
                        nops.append(nop)
                    si.on_wait = keep
                    il[i:i] = nops
                    i += len(nops)
                    n_split += 1
                i += 1
    return n_split


class _SlimTileContext(tile.TileContext):
    """TileContext whose kernel tail skips the trailing all-engine barrier.

    The drain instruction already waits on the full vector clock (all
    compute + DMA completions) and the first barrier synchronizes every
    engine behind it. With BASS_KERNEL_TAIL_CLEARS set, semaphores are
    cleared for safe re-execution (needed when timing repeated runs).
    """

    def _drain_and_barrier(self, tick_clock, wait_clock):
        from concourse.tile import ScopedClock

        drain_inst = self.nc.sync.drain()
        wait_clock.add_sem_waits(
            drain_inst.ins, ScopedClock({None: tick_clock.global_clock})
        )
        self.nc.all_engine_barrier()
        popped = self.nc._tile_sem_poison_stack.pop()
        assert popped is self._sem_poison
        if os.environ.get("BASS_KERNEL_TAIL_CLEARS"):
            self.nc.clear_and_free_semaphores(list(self.sems.allocated().values()))


def _segments(caps):
    """Static segment layout helpers for one core."""
    bounds = np.cumsum(np.asarray(caps, dtype=np.int64))

    def expert_at(pos):
        return int(np.searchsorted(bounds, pos, side="right"))

    def runs(s):
        # expert runs within the 512-token block starting at s
        out = []
        pos = s
        while pos < s + 512:
            e = expert_at(pos)
            end = min(s + 512, int(bounds[e]))
            out.append((pos - s, end - pos, e))
            pos = end
        return out

    return expert_at, runs


def _pair_plan(caps):
    """Compile-time plan: per 1024-token pair, the matmul runs for the top
    (partitions 0:64) and bottom (64:128) halves, plus the bias subruns
    with their (expert_top, expert_bottom) combo index. A trailing lone
    512-block (ntot % 1024 == 512) gets rb=None and top-only subruns."""
    ntot = int(sum(caps))
    expert_at, runs = _segments(caps)
    combos = []
    combo_idx = {}

    def cidx(key):
        if key not in combo_idx:
            combo_idx[key] = len(combos)
            combos.append(key)
        return combo_idx[key]

    plan = []
    for k in range((ntot + 1023) // 1024):
        s = 1024 * k
        rt = runs(s)
        if s + 1024 <= ntot:
            rb = runs(s + 512)
            cuts = sorted(
                {0, 512} | {o for (o, _, _) in rt} | {o for (o, _, _) in rb}
            )
            subs = [
                (o1, o2 - o1, cidx((expert_at(s + o1), expert_at(s + 512 + o1))))
                for o1, o2 in zip(cuts[:-1], cuts[1:])
            ]
        else:
            rb = None
            subs = [(o, n, cidx((e, e))) for (o, n, e) in rt]
        plan.append((rt, rb, subs))
    return plan, combos


def _build_program(caps):
    """Bass program for one core: segmented fp16 matmul over pre-sorted xT.

    caps: tuple of per-expert segment capacities (tokens, multiples of 32);
    their sum (ntot) is a multiple of 1024. Segment boundaries are static.
    """
    ntot = int(sum(caps))
    assert ntot % 512 == 0
    lone = (ntot % 1024) == 512
    plan, combos = _pair_plan(caps)
    NC = len(combos)

    # groups of GROUP tokens, with a 512-multiple tail
    groups = []
    pos = 0
    while pos < ntot:
        gl = min(GROUP, ntot - pos)
        groups.append((pos, gl))
        pos += gl

    nc = bass.Bass(trn_type="TRN2")
    dt = mybir.dt
    NB2 = (ntot + 1023) // 1024    # stacked 512-col blocks (pairs + lone)
    xT = nc.declare_dram_parameter("xT", [D, ntot], dt.float16, isOutput=False)
    Wp = nc.declare_dram_parameter("Wp", [128, E * 2 * P], dt.float16, isOutput=False)
    bT = nc.declare_dram_parameter("bT", [128, NC], dt.float32, isOutput=False)
    ys2 = nc.declare_dram_parameter("ys2", [128, NB2 * 512], dt.float16, isOutput=True)

    def load_pieces(gof, gl):
        # 1 MiB pieces; the stream tail in 1024-col pieces so the final
        # compute+store chain hangs off a small last transfer
        ps_, s = [], 0
        while s < gl:
            pl = min(PIECE, gl - s)
            if gof + s + pl == ntot and pl > 1024:
                for s2 in range(s, s + pl, 1024):
                    ps_.append((s2, min(1024, s + pl - s2)))
            else:
                ps_.append((s, pl))
            s += pl
        return ps_

    with _SlimTileContext(nc) as tc:
        with (
            tc.tile_pool(name="consts", bufs=1) as consts,
            tc.tile_pool(name="xtp", bufs=3) as xtp,
            tc.tile_pool(name="yp", bufs=3) as yp,
            tc.tile_pool(name="ps", bufs=8, space="PSUM") as ps,
        ):
            # weights + bias in single DMAs on the scalar ring, in parallel
            # with the first x piece on the sync ring
            wt = consts.tile([128, E * 2 * P], dt.float16)
            bt = consts.tile([128, NC], dt.float32)
            nc.scalar.dma_start(wt[:], Wp[:])
            nc.scalar.dma_start(bt[:], bT[:])

            pair_i = 0
            for gi, (gof, gl) in enumerate(groups):
                xt0 = xtp.tile([128, GROUP], dt.float16, tag="xt0")
                xt1 = xtp.tile([128, GROUP], dt.float16, tag="xt1")
                for s, pl in load_pieces(gof, gl):
                    nc.sync.dma_start(
                        xt0[:, s : s + pl], xT[0:128, gof + s : gof + s + pl]
                    )
                    nc.sync.dma_start(
                        xt1[:, s : s + pl], xT[128:256, gof + s : gof + s + pl]
                    )

                yts = yp.tile([128, GROUP // 2], dt.float16, tag="yts")
                for kp in range((gl + 1023) // 1024):
                    rt, rb, subs = plan[pair_i]
                    pair_i += 1
                    q = kp * 1024          # token offset within group
                    yq = kp * 512          # stacked col offset within yts
                    pt = ps.tile([128, 512], dt.float32, tag="pt")
                    halves = ((0, rt, q),) if rb is None else (
                        (0, rt, q), (64, rb, q + 512))
                    for base, rr, xoff in halves:
                        for off, n, e in rr:
                            nc.tensor.matmul(
                                pt[base : base + 64, off : off + n],
                                lhsT=wt[:, (e * 2 + 0) * P : (e * 2 + 1) * P],
                                rhs=xt0[:, xoff + off : xoff + off + n],
                                start=True,
                                stop=False,
                            )
                            nc.tensor.matmul(
                                pt[base : base + 64, off : off + n],
                                lhsT=wt[:, (e * 2 + 1) * P : (e * 2 + 2) * P],
                                rhs=xt1[:, xoff + off : xoff + off + n],
                                start=False,
                                stop=True,
                            )
                    # bias add doubles as the PSUM->SBUF copy; alternate
                    # engines so neither becomes the bottleneck
                    pb = 64 if rb is None else 128
                    for off, n, j in subs:
                        if pair_i % 2:
                            nc.vector.tensor_scalar_add(
                                yts[:pb, yq + off : yq + off + n],
                                pt[:pb, off : off + n],
                                bt[:pb, j : j + 1],
                            )
                        else:
                            nc.scalar.add(
                                yts[:pb, yq + off : yq + off + n],
                                pt[:pb, off : off + n],
                                bt[:pb, j : j + 1],
                            )
                # stream stores in pieces so no single store issues late;
                # per-block pieces at the stream tail to shrink the chain
                gcols = ((gl + 1023) // 1024) * 512  # stacked cols this group
                sbase = (gof // 1024) * 512
                is_last = gof + gl == ntot
                pieces = []
                s = 0
                while s < gcols:
                    near_end = is_last and s >= gcols - 2048
                    pl = min(512 if near_end else 2048, gcols - s)
                    pieces.append((s, pl))
                    s += pl
                for s, pl in pieces:
                    lone_piece = lone and is_last and s + pl == gcols
                    ph = 64 if lone_piece else 128
                    nc.scalar.dma_start(
                        ys2[:ph, sbase + s : sbase + s + pl],
                        yts[:ph, s : s + pl],
                    )

    return nc, combos


_cache = {"key": None, "nc": None, "combos": None}
last_exec_time_ns = None
_last_run = None


def kernel(x, W, b, block_indices):
    global last_exec_time_ns, _last_run
    x = np.asarray(x, dtype=np.float32)
    W = np.asarray(W, dtype=np.float32)
    b = np.asarray(b, dtype=np.float32)
    sel = np.asarray(block_indices).astype(np.int64).reshape(-1)
    x_flat = x.reshape(B * T, D).astype(np.float16)

    # routing is per-token, so token->core assignment is free: deal each
    # expert's tokens evenly across cores. All cores then have near-identical
    # per-expert counts (no straggler core, minimal shared-layout padding).
    ids = [[None] * E for _ in range(N_CORES)]
    counts = np.zeros((N_CORES, E), dtype=np.int64)
    for e in range(E):
        ge = np.flatnonzero(sel == e)
        parts = np.array_split(ge, N_CORES)
        for c in range(N_CORES):
            ids[c][e] = parts[c]
            counts[c, e] = len(parts[c])

    # shared static segment layout: capacity per expert = max over cores,
    # rounded up to 2 (4B fp16 alignment); total rounded up to 512
    caps = ((counts.max(axis=0) + 1) // 2 * 2).astype(np.int64)
    ntot = int(((caps.sum() + 511) // 512) * 512)
    caps[E - 1] += ntot - caps.sum()
    offs = np.concatenate([[0], np.cumsum(caps)])

    key = tuple(int(cp) for cp in caps)
    if _cache["key"] != key:
        nc, combos = _build_program(key)
        _split_multi_waits(nc)
        _cache["nc"] = nc
        _cache["combos"] = combos
        _cache["key"] = key
    combos = _cache["combos"]

    # weights: [E, D, P] -> [128, E*2*P] fp16 tiles (K-half h of expert e at
    # columns (e*2+h)*P); bias as fp32 per-combo stacked columns [128, NC]
    Wp = np.ascontiguousarray(
        W.reshape(E, 2, 128, P).transpose(2, 0, 1, 3).reshape(128, E * 2 * P)
    ).astype(np.float16)
    bT = np.empty((128, len(combos)), dtype=np.float32)
    for j, (eT, eB) in enumerate(combos):
        bT[:64, j] = b[eT]
        bT[64:, j] = b[eB]

    in_maps = []
    for c in range(N_CORES):
        # padded sorted order; pad slots replay token 0 (results discarded)
        po = np.zeros(ntot, dtype=np.int64)
        for e in range(E):
            po[offs[e] : offs[e] + counts[c, e]] = ids[c][e]
        xT = np.ascontiguousarray(x_flat[po].T)
        in_maps.append({"xT": xT, "Wp": Wp, "bT": bT})

    trace = bool(os.environ.get("BASS_KERNEL_TRACE"))
    res = run_bass_kernel_spmd(
        _cache["nc"], in_maps, list(range(N_CORES)), trace=trace
    )
    last_exec_time_ns = res.exec_time_ns
    _last_run = {"nc": _cache["nc"], "in_maps": in_maps}

    out_flat = np.empty((B * T, P), dtype=np.float32)
    npairs = ntot // 1024
    lone = (ntot % 1024) == 512
    for c in range(N_CORES):
        y2 = res.results[c]["ys2"]  # [128, NB2*512] fp16, stacked pairs
        ys = np.empty((ntot, P), dtype=np.float32)
        yp = y2[:, : npairs * 512]
        ysr = ys[: npairs * 1024].reshape(npairs, 2, 512, P)
        ysr[:, 0] = yp[:64].reshape(64, npairs, 512).transpose(1, 2, 0)
        ysr[:, 1] = yp[64:].reshape(64, npairs, 512).transpose(1, 2, 0)
        if lone:
            ys[npairs * 1024 :] = y2[:64, npairs * 512 : npairs * 512 + 512].T
        for e in range(E):
            out_flat[ids[c][e]] = ys[offs[e] : offs[e] + counts[c, e]]
    return out_flat.reshape(B, T, P)
